# revision 1
# baseline (speedup 1.0000x reference)
"""Trainium2 Bass kernel for a pre-LN multi-head attention block.

Full-input contract: kernel(**inputs) takes the unsharded tensors from
setup_inputs() and returns the full [4, 2048, 1024] output.

Sharding: 8 cores = 4 batches x 2 head-groups (8 heads each).
Each core computes LayerNorm(x[b]) (replicated within the batch pair),
its 8 heads of QKV + attention, and a partial projection
(attn_out_part @ w_proj_rows).  Host sums the two partials per batch and
adds b_proj + residual.

Host-side algebraic folds (exact):
  - ln_w folded into w_qkv columns, ln_b folded into b_qkv
  - softmax scale (0.125, exact in fp32/bf16) folded into W_q / b_q

On-core dataflow (all layouts chosen so nothing but h needs transposing):
  LN:    x[128tok,1024] -> h bf16, PE-transpose -> hT [emb, tok]
  QKV:   QT/KT = (Wq|Wk)^T hT  -> [head_dim*2heads, tok] tiles
         V    = hT^T Wv (+ones-row bias matmul) -> V_aug [tok, 8x(64+1)]
  Attn:  ST[k,q] = KT^T-slice x QT-slice (contract d=64)
         expST = Exp(ST) on ACT -> bf16
         OT_aug[65, q] += V_aug^T @ expST   (row 64 = softmax sums)
         normalize: recip(ones^T@sums bcast) * OT -> OTn bf16
  Proj:  Z[tok, emb] = OTn^T-slices @ Wp rows, accumulated over 4 pairs
"""

import sys

sys.path.insert(0, "/opt/trn_rl_repo")

import numpy as np
import ml_dtypes

import concourse.bass as bass
from concourse import bacc
import concourse.tile as tile
from concourse import mybir
from concourse.bass_utils import run_bass_kernel_spmd
from concourse.masks import make_identity

EMB = 1024
HEADS = 16
HD = 64
SCALE = HD ** -0.5
N_TOK = 2048
N_CORES = 8
HPC = 8                 # heads per core
QK_COLS = HPC * HD      # 512
P = 128
NT = N_TOK // P         # 16 token tiles
EC = EMB // P           # 8 emb chunks
QCH = 4                 # q chunks of 512
NKT = 16                # k tiles of 128
NPAIR = HPC // 2        # 4 head-pair tiles

BF16 = mybir.dt.bfloat16
F32 = mybir.dt.float32
AF = mybir.ActivationFunctionType


def build_nc():
    nc = bacc.Bacc(trn_type="TRN2", target_bir_lowering=False)

    x_d = nc.dram_tensor("x", [N_TOK, EMB], F32, kind="ExternalInput")
    wq_d = nc.dram_tensor("wq", [EMB // 2, 2 * QK_COLS], BF16, kind="ExternalInput")
    wk_d = nc.dram_tensor("wk", [EMB // 2, 2 * QK_COLS], BF16, kind="ExternalInput")
    wv_d = nc.dram_tensor("wv", [EMB // 2, 2 * QK_COLS], BF16, kind="ExternalInput")
    bqt_d = nc.dram_tensor("bqt", [P, NPAIR], F32, kind="ExternalInput")
    bkt_d = nc.dram_tensor("bkt", [P, NPAIR], F32, kind="ExternalInput")
    bv_d = nc.dram_tensor("bv", [1, QK_COLS], BF16, kind="ExternalInput")
    wp_d = nc.dram_tensor("wp", [QK_COLS // 2, 2 * EMB], BF16, kind="ExternalInput")
    z_d = nc.dram_tensor("z", [N_TOK, EMB], F32, kind="ExternalOutput")

    with tile.TileContext(nc) as tc:
        _emit(nc, tc, x_d, wq_d, wk_d, wv_d, bqt_d, bkt_d, bv_d, wp_d, z_d)
    nc.finalize()
    return nc


def _emit(nc, tc, x_d, wq_d, wk_d, wv_d, bqt_d, bkt_d, bv_d, wp_d, z_d):
    from contextlib import ExitStack

    ctx = ExitStack()
    with ctx:
        consts = ctx.enter_context(tc.tile_pool(name="consts", bufs=1))
        persist = ctx.enter_context(tc.tile_pool(name="persist", bufs=1))

        ident = consts.tile([P, P], BF16, tag="ident", name="ident")
        make_identity(nc, ident)
        ones_row = consts.tile([1, P], BF16, tag="ones_row", name="ones_row")
        nc.vector.memset(ones_row, 1.0)
        ones64 = consts.tile([1, HD], BF16, tag="ones64", name="ones64")
        nc.vector.memset(ones64, 1.0)
        eps_t = consts.tile([P, 1], F32, tag="eps", name="eps")
        nc.vector.memset(eps_t, 1e-5)

        bqt = consts.tile([P, NPAIR], F32, tag="bqt", name="bqt")
        nc.sync.dma_start(out=bqt, in_=bqt_d[:, :])
        bkt = consts.tile([P, NPAIR], F32, tag="bkt", name="bkt")
        nc.sync.dma_start(out=bkt, in_=bkt_d[:, :])
        bvt = consts.tile([1, QK_COLS], BF16, tag="bvt", name="bvt")
        nc.sync.dma_start(out=bvt, in_=bv_d[:, :])

        wq_s = []
        wk_s = []
        wv_s = []
        for c in range(EC // 2):
            for lst, srcd, nm in ((wq_s, wq_d, "wq"), (wk_s, wk_d, "wk"),
                                  (wv_s, wv_d, "wv")):
                t = persist.tile([P, 2, QK_COLS], BF16, tag=f"{nm}{c}", name=f"{nm}{c}")
                nc.sync.dma_start(out=t, in_=srcd[c * P:(c + 1) * P, :].rearrange(
                    "p (r m) -> p r m", r=2))
                lst.append(t)
        wp_s = []
        for i in range(2):
            t = persist.tile([P, 2, EMB], BF16, tag=f"wp{i}", name=f"wp{i}")
            nc.sync.dma_start(out=t, in_=wp_d[i * P:(i + 1) * P, :].rearrange(
                "p (r m) -> p r m", r=2))
            wp_s.append(t)

        qt = [persist.tile([P, N_TOK], BF16, tag=f"qt{i}", name=f"qt{i}") for i in range(NPAIR)]
        kt = [persist.tile([P, N_TOK], BF16, tag=f"kt{i}", name=f"kt{i}") for i in range(NPAIR)]
        otn = [persist.tile([P, 2, N_TOK], BF16, tag=f"otn{i}", name=f"otn{i}") for i in range(2)]
        vaug = [persist.tile([P, 2, HPC, HD + 1], BF16, tag=f"vaug{i}", name=f"vaug{i}")
                for i in range(NT // 2)]
        for t in range(NT // 2):
            nc.vector.memset(vaug[t][:, :, :, HD:HD + 1], 1.0)

        # ---------------- Phase 1: LayerNorm + transpose ----------------
        ht_ctx = ExitStack()
        ht_pool = ht_ctx.enter_context(tc.tile_pool(name="ht", bufs=1))
        ht = [ht_pool.tile([P, 2, N_TOK], BF16, tag=f"ht{e}", name=f"ht{e}") for e in range(EC // 2)]

        with tc.tile_pool(name="ln", bufs=3) as ln_pool, \
             tc.tile_pool(name="lns", bufs=6) as lns, \
             tc.tile_pool(name="ps_tr", bufs=3, space="PSUM") as ps_tr:
            for t in range(NT):
                x_t = ln_pool.tile([P, EMB], F32, tag="x", name="x")
                nc.sync.dma_start(out=x_t, in_=x_d[t * P:(t + 1) * P, :])
                stats = lns.tile([P, 2, 6], F32, tag="stats", name="stats")
                nc.vector.bn_stats(out=stats[:, 0, :], in_=x_t[:, 0:512])
                nc.vector.bn_stats(out=stats[:, 1, :], in_=x_t[:, 512:1024])
                mv = lns.tile([P, 2], F32, tag="mv", name="mv")
                nc.vector.bn_aggr(out=mv, in_=stats)
                sd = lns.tile([P, 1], F32, tag="sd", name="sd")
                nc.scalar.activation(out=sd, in_=mv[:, 1:2], func=AF.Sqrt,
                                     bias=eps_t, scale=1.0)
                rstd = lns.tile([P, 1], F32, tag="rstd", name="rstd")
                nc.vector.reciprocal(out=rstd, in_=sd)
                nmean = lns.tile([P, 1], F32, tag="nmean", name="nmean")
                nc.vector.tensor_scalar_mul(nmean, mv[:, 0:1], -1.0)
                xc = ln_pool.tile([P, EMB], F32, tag="xc", name="xc")
                nc.vector.tensor_scalar_add(xc, x_t, nmean)
                h_t = ln_pool.tile([P, EMB], BF16, tag="h", name="h")
                nc.vector.tensor_scalar_mul(h_t, xc, rstd)
                for e in range(EC):
                    pt = ps_tr.tile([P, P], BF16, tag="tr", name="tr")
                    nc.tensor.transpose(pt, h_t[:, e * P:(e + 1) * P], ident)
                    nc.vector.tensor_copy(
                        out=ht[e // 2][:, e % 2, t * P:(t + 1) * P], in_=pt)

        # ---------------- Phase 2: QKV matmuls ----------------
        with tc.tile_pool(name="ps_qkv", bufs=3, space="PSUM") as ps_qkv:
            for m in range(NPAIR):
                for n in range(QCH):
                    pq = ps_qkv.tile([P, 512], F32, tag="qkv", name="qkv")
                    for c in range(EC // 2):
                      for r in range(2):
                        nc.tensor.matmul(pq, lhsT=wq_s[c][:, r, m * P:(m + 1) * P],
                                         rhs=ht[c][:, r, n * 512:(n + 1) * 512],
                                         start=(c == 0 and r == 0),
                                         stop=(c == EC // 2 - 1 and r == 1))
                    nc.scalar.activation(out=qt[m][:, n * 512:(n + 1) * 512],
                                         in_=pq, func=AF.Identity,
                                         bias=bqt[:, m:m + 1], scale=1.0)
                    pk = ps_qkv.tile([P, 512], F32, tag="qkv", name="qkv")
                    for c in range(EC // 2):
                      for r in range(2):
                        nc.tensor.matmul(pk, lhsT=wk_s[c][:, r, m * P:(m + 1) * P],
                                         rhs=ht[c][:, r, n * 512:(n + 1) * 512],
                                         start=(c == 0 and r == 0),
                                         stop=(c == EC // 2 - 1 and r == 1))
                    nc.scalar.activation(out=kt[m][:, n * 512:(n + 1) * 512],
                                         in_=pk, func=AF.Identity,
                                         bias=bkt[:, m:m + 1], scale=1.0)
            for t in range(NT):
                pv = ps_qkv.tile([P, 512], F32, tag="qkv", name="qkv")
                for c in range(EC // 2):
                  for r in range(2):
                    nc.tensor.matmul(pv, lhsT=ht[c][:, r, t * P:(t + 1) * P],
                                     rhs=wv_s[c][:, r, :],
                                     start=(c == 0 and r == 0), stop=False)
                nc.tensor.matmul(pv, lhsT=ones_row, rhs=bvt,
                                 start=False, stop=True)
                nc.vector.tensor_copy(
                    out=vaug[t // 2][:, t % 2, :, 0:HD],
                    in_=pv.rearrange("p (h d) -> p h d", h=HPC))

        ht_ctx.close()

        # ---------------- Phase 3: attention ----------------
        with tc.tile_pool(name="expp", bufs=20) as expp, \
             tc.tile_pool(name="att_sm", bufs=4) as att_sm, \
             tc.tile_pool(name="ps_st", bufs=2, space="PSUM") as ps_st, \
             tc.tile_pool(name="ps_ot", bufs=2, space="PSUM") as ps_ot, \
             tc.tile_pool(name="ps_b", bufs=1, space="PSUM") as ps_b:
            def emit_st(h, q):
                """Scores + exp for one (head, q-chunk); returns exp tiles."""
                pair, row = divmod(h, 2)
                row *= HD
                ets = []
                for c in range(NKT // 2):
                    pst = ps_st.tile([P, 2, 512], F32, tag="st", name="st")
                    for r in range(2):
                        k = 2 * c + r
                        nc.tensor.matmul(
                            pst[:, r, :],
                            lhsT=kt[pair][row:row + HD, k * P:(k + 1) * P],
                            rhs=qt[pair][row:row + HD, q * 512:(q + 1) * 512],
                            start=True, stop=True)
                    e_t = expp.tile([P, 2, 512], BF16, tag="e", name="e")
                    nc.scalar.activation(out=e_t, in_=pst, func=AF.Exp)
                    ets.append(e_t)
                return ets

            def emit_av(h, q, ets):
                """att@v + normalize for one (head, q-chunk)."""
                pot = ps_ot.tile([HD + 1, 512], F32, tag="ot", name="ot")
                for k in range(NKT):
                    nc.tensor.matmul(pot, lhsT=vaug[k // 2][:, k % 2, h, :],
                                     rhs=ets[k // 2][:, k % 2, :],
                                     start=(k == 0), stop=(k == NKT - 1))
                srow = att_sm.tile([1, 512], BF16, tag="srow", name="srow")
                nc.vector.tensor_copy(out=srow, in_=pot[HD:HD + 1, :])
                pb = ps_b.tile([HD, 512], F32, tag="b", name="b")
                nc.tensor.matmul(pb, lhsT=ones64, rhs=srow,
                                 start=True, stop=True)
                rec = att_sm.tile([HD, 512], F32, tag="rec", name="rec")
                nc.vector.reciprocal(out=rec, in_=pb)
                nc.vector.tensor_mul(
                    otn[h // 4][(h % 2) * HD:(h % 2) * HD + HD, (h // 2) % 2,
                                q * 512:(q + 1) * 512],
                    pot[0:HD, :], rec)

            prev = None
            for h in range(HPC):
                for q in range(QCH):
                    ets = emit_st(h, q)
                    if prev is not None:
                        emit_av(*prev)
                    prev = (h, q, ets)
            emit_av(*prev)

        # ---------------- Phase 4: projection ----------------
        with tc.tile_pool(name="ps_z", bufs=3, space="PSUM") as ps_z, \
             tc.tile_pool(name="zst", bufs=3) as zst:
            for t in range(NT):
                for ec2 in range(2):
                    pz = ps_z.tile([P, 512], F32, tag="z", name="z")
                    for c in range(2):
                      for r in range(2):
                        nc.tensor.matmul(
                            pz, lhsT=otn[c][:, r, t * P:(t + 1) * P],
                            rhs=wp_s[c][:, r, ec2 * 512:(ec2 + 1) * 512],
                            start=(c == 0 and r == 0), stop=(c == 1 and r == 1))
                    z_t = zst.tile([P, 512], F32, tag="z", name="z")
                    nc.vector.tensor_copy(out=z_t, in_=pz)
                    nc.sync.dma_start(
                        out=z_d[t * P:(t + 1) * P, ec2 * 512:(ec2 + 1) * 512],
                        in_=z_t)


_CACHE = {}


def _get_nc():
    if "nc" not in _CACHE:
        _CACHE["nc"] = build_nc()
    return _CACHE["nc"]


def _prep_in_maps(x, ln_w, ln_b, w_qkv, b_qkv, w_proj, b_proj):
    bf = ml_dtypes.bfloat16
    x = np.asarray(x, np.float32)
    ln_w = np.asarray(ln_w, np.float32)
    ln_b = np.asarray(ln_b, np.float32)
    w_qkv = np.asarray(w_qkv, np.float32)
    b_qkv = np.asarray(b_qkv, np.float32)
    w_proj = np.asarray(w_proj, np.float32)

    b_eff = b_qkv + ln_b @ w_qkv
    w_eff = ln_w[:, None] * w_qkv
    w4 = w_eff.reshape(EMB, HEADS, HD, 3)
    b4 = b_eff.reshape(HEADS, HD, 3)
    wq = w4[..., 0] * SCALE
    wk = w4[..., 1]
    wv = w4[..., 2]
    bq = b4[..., 0] * SCALE
    bk = b4[..., 1]
    bv = b4[..., 2]

    def _dr(w):
        # [R, M] -> [R/2, 2M]: row 256c+128r+k -> (c*128+k, r*M+m)
        R, M = w.shape
        return np.ascontiguousarray(
            w.reshape(R // 256, 2, 128, M).transpose(0, 2, 1, 3).reshape(R // 2, 2 * M))

    def bias_t(b, hsl):
        # [8, 64] head-slice -> [512] (pair-major) -> [128, 4] transposed
        v = b[hsl].reshape(QK_COLS)
        return np.ascontiguousarray(v.reshape(NPAIR, P).T.astype(np.float32))

    in_maps = []
    for cid in range(N_CORES):
        bi, hg = divmod(cid, 2)
        hsl = slice(hg * HPC, (hg + 1) * HPC)
        in_maps.append({
            "x": np.ascontiguousarray(x[bi]),
            "wq": _dr(wq[:, hsl, :].reshape(EMB, QK_COLS)).astype(bf),
            "wk": _dr(wk[:, hsl, :].reshape(EMB, QK_COLS)).astype(bf),
            "wv": _dr(wv[:, hsl, :].reshape(EMB, QK_COLS)).astype(bf),
            "bqt": bias_t(bq, hsl),
            "bkt": bias_t(bk, hsl),
            "bv": np.ascontiguousarray(
                bv[hsl].reshape(1, QK_COLS)).astype(bf),
            "wp": _dr(w_proj[hg * QK_COLS:(hg + 1) * QK_COLS, :]).astype(bf),
        })
    return in_maps


def _gather(results, x, b_proj):
    b_proj = np.asarray(b_proj, np.float32)
    x = np.asarray(x, np.float32)
    out = np.empty((x.shape[0], N_TOK, EMB), np.float32)
    for bi in range(x.shape[0]):
        out[bi] = (results[2 * bi]["z"] + results[2 * bi + 1]["z"]
                   + b_proj[None, :] + x[bi])
    return out


def _run(inputs, **kw):
    in_maps = _prep_in_maps(**inputs)
    res = run_bass_kernel_spmd(_get_nc(), in_maps,
                               core_ids=list(range(N_CORES)), **kw)
    out = _gather(res.results, inputs["x"], inputs["b_proj"])
    return out, res


def kernel(**inputs):
    out, _ = _run(inputs)
    return out

